# revision 8
# baseline (speedup 1.0000x reference)
"""Multi-head causal attention (B=4,T=2048,C=768,H=12,HS=64) on 8 trn2 cores.

Sharding: core c -> (batch b=c//2, head-half s=c%2, heads 6s..6s+5).
Each core computes QKV + attention for its 6 heads over the full T, then the
cores of a batch exchange attention outputs via a world AllGather and each
computes the full output projection for its batch.  Host keeps core 2b's
output for batch b.

All matmuls in bf16 (PSUM accumulates fp32).  Scores are computed
transposed ([tk, tq]) so softmax denominators come from a ones-column in V
and no PE transposes of the TxT attention matrix are needed.
"""
import sys

if "/opt/trn_rl_repo" not in sys.path:
    sys.path.insert(0, "/opt/trn_rl_repo")

import numpy as np
import ml_dtypes

import concourse.bass as bass
import concourse.mybir as mybir
import concourse.tile as tile
from concourse import bacc
from concourse.bass import ts, ds
from concourse.bass_utils import run_bass_kernel_spmd
from concourse.masks import make_identity

B, T_FULL, C, H, HS = 4, 2048, 768, 12, 64
N_CORES = 8
P = 128
NCH = C // P  # 6 c-chunks
dt = mybir.dt
bf16 = ml_dtypes.bfloat16
F32 = dt.float32
BF = dt.bfloat16


def build(T=T_FULL):
    nq = T // 512          # number of 512-wide q ranges
    nkt = T // P           # number of 128-wide k tiles
    nc = bacc.Bacc("TRN2", target_bir_lowering=False, debug=False,
                   num_devices=N_CORES)
    xt_d = nc.dram_tensor("xt", [C, T], BF, kind="ExternalInput").ap()
    wq_d = nc.dram_tensor("wq", [C, 384], BF, kind="ExternalInput").ap()
    wk_d = nc.dram_tensor("wk", [C, 384], BF, kind="ExternalInput").ap()
    wv_d = nc.dram_tensor("wv", [C, 384], BF, kind="ExternalInput").ap()
    wpt_d = nc.dram_tensor("wpt", [C, C], BF, kind="ExternalInput").ap()
    bp_d = nc.dram_tensor("bp", [1, C], BF, kind="ExternalInput").ap()
    meta_d = nc.dram_tensor("meta", [64, 6], dt.int32, kind="ExternalInput").ap()
    out_d = nc.dram_tensor("out", [T, C], F32, kind="ExternalOutput").ap()

    with tile.TileContext(nc) as tc:
        with tc.tile_pool(name="const", bufs=1) as const, \
             tc.tile_pool(name="big", bufs=1) as big, \
             tc.tile_pool(name="sbp", bufs=2) as sbp, \
             tc.tile_pool(name="ptp", bufs=3) as ptp, \
             tc.tile_pool(name="pp", bufs=2, space="PSUM") as pp, \
             tc.tile_pool(name="psc", bufs=2, space="PSUM") as psc, \
             tc.tile_pool(name="pso", bufs=2, space="PSUM") as pso, \
             tc.tile_pool(name="dram", bufs=1, space="DRAM") as dram:

            # ---- persistent loads ----
            xt = big.tile([P, NCH, T], BF, tag="xt")
            nc.sync.dma_start(xt[:], xt_d.rearrange("(n p) m -> p n m", p=P))
            wq = big.tile([P, NCH, 3, P], BF, tag="wq")
            wk = big.tile([P, NCH, 3, P], BF, tag="wk")
            wv = big.tile([P, NCH, 3, P], BF, tag="wv")
            for wt, wd in ((wq, wq_d), (wk, wk_d), (wv, wv_d)):
                nc.sync.dma_start(
                    wt[:], wd.rearrange("(n p) (r m) -> p n r m", p=P, m=P))
            wpt = big.tile([64, 12, C], BF, tag="wpt")
            nc.sync.dma_start(wpt[:], wpt_d.rearrange("(j p) m -> p j m", p=64))
            bp_sb = const.tile([1, C], BF, tag="bp")
            nc.sync.dma_start(bp_sb[:], bp_d[:])
            meta_sb = const.tile([64, 6], dt.int32, tag="meta")
            nc.sync.dma_start(meta_sb[:], meta_d[:])
            ones1 = const.tile([1, P], BF, tag="ones1")
            nc.gpsimd.memset(ones1[:], 1.0)
            ident = const.tile([P, P], BF, tag="ident")
            make_identity(nc, ident[:])

            att_all = big.tile([64, 6, T], BF, tag="att_all")
            att_oth = big.tile([64, 6, T], BF, tag="att_oth")

            EXP = mybir.ActivationFunctionType.Exp

            # ---- per head-pair: QKV + attention ----
            for r in range(3):
                qt = sbp.tile([P, T], BF, tag="qt")
                kt_t = sbp.tile([P, T], BF, tag="kt")
                vt_t = sbp.tile([P, T], BF, tag="vt")
                for wt, dst in ((wq, qt), (wk, kt_t), (wv, vt_t)):
                    for tr in range(nq):
                        ps = pp.tile([P, 512], F32, tag="qkv")
                        for ci in range(NCH):
                            nc.tensor.matmul(ps[:], wt[:, ci, r, :],
                                             xt[:, ci, ts(tr, 512)],
                                             start=(ci == 0), stop=(ci == NCH - 1))
                        nc.vector.tensor_copy(dst[:, ts(tr, 512)], ps[:])
                # V natural (+ ones col) per head
                vaug0 = sbp.tile([P, nkt, 65], BF, tag="vaug0")
                vaug1 = sbp.tile([P, nkt, 65], BF, tag="vaug1")
                nc.gpsimd.memset(vaug0[:, :, 64:65], 1.0)
                nc.gpsimd.memset(vaug1[:, :, 64:65], 1.0)
                for k in range(nkt):
                    pst = pp.tile([P, P], BF, tag="qkv")
                    nc.tensor.transpose(pst[:], vt_t[:, ts(k, P)], ident[:])
                    nc.vector.tensor_copy(vaug0[:, k, 0:64], pst[:, 0:64])
                    nc.vector.tensor_copy(vaug1[:, k, 0:64], pst[:, 64:128])
                vaug = (vaug0, vaug1)

                # attention per q-range, heads a=0,1 interleaved
                for qr in range(nq):
                    n_k = 4 * qr + 4       # causal: k tiles 0..4qr+3
                    oT0 = pso.tile([65, 512], F32, tag="oT")
                    oT1 = pso.tile([65, 512], F32, tag="oT")
                    oT = (oT0, oT1)
                    for k0 in range(0, n_k, 2):
                        scs, offs = [], []
                        for a in (0, 1):
                            sc = psc.tile([P, 2, 512], F32, tag="sc")
                            lo = 128 * max(0, k0 - 4 * qr)  # lowest col offset
                            for dk in (0, 1):
                                kti = k0 + dk
                                # cover from the pair's low offset so the
                                # whole exp'd region is written (extra cols
                                # are masked region, never read by PV)
                                nc.tensor.matmul(
                                    sc[:, dk, lo:512],
                                    kt_t[64 * a:64 * a + 64, ts(kti, P)],
                                    qt[64 * a:64 * a + 64,
                                       qr * 512 + lo:(qr + 1) * 512],
                                    start=True, stop=True)
                            scs.append(sc)
                            offs.append(lo)
                        pts = []
                        for a in (0, 1):
                            lo = offs[a]
                            pt = ptp.tile([P, 2, 512], BF, tag="pt")
                            nc.scalar.activation(pt[:, :, lo:512],
                                                 scs[a][:, :, lo:512], EXP)
                            for dk in (0, 1):
                                m = k0 + dk - 4 * qr
                                if 0 <= m < 4:
                                    blk = pt[:, dk, 128 * m:128 * (m + 1)]
                                    nc.gpsimd.affine_select(
                                        out=blk, in_=blk,
                                        compare_op=mybir.AluOpType.is_ge,
                                        fill=0.0, base=0,
                                        pattern=[[1, P]], channel_multiplier=-1)
                            pts.append(pt)
                        for a in (0, 1):
                            for dk in (0, 1):
                                kti = k0 + dk
                                off = 128 * max(0, kti - 4 * qr)
                                nc.tensor.matmul(
                                    oT[a][:, off:512],
                                    vaug[a][:, kti, :],
                                    pts[a][:, dk, off:512],
                                    start=(kti == 0), stop=(kti == n_k - 1))
                    for a in (0, 1):
                        r_sb = sbp.tile([1, 512], F32, tag="r")
                        nc.vector.reciprocal(r_sb[:], oT[a][64:65, :])
                        rb = sbp.tile([64, 512], F32, tag="rb")
                        nc.gpsimd.partition_broadcast(rb[:], r_sb[:])
                        nc.vector.tensor_mul(
                            att_all[:, 2 * r + a, ts(qr, 512)],
                            oT[a][0:64, :], rb[:])

            # ---- exchange attT halves via world AllGather ----
            # in_cc row index = p*6 + j so one indirect gather (64 rows of
            # 6*T) fetches the partner slab; indices are host-fed via meta.
            in_cc = dram.tile([384, T], BF, tag="in_cc")
            out_cc = dram.tile([N_CORES * 384, T], BF, tag="out_cc")
            nc.sync.dma_start(in_cc.rearrange("(p j) m -> p j m", j=6),
                              att_all[:])
            nc.gpsimd.collective_compute(
                "AllGather", mybir.AluOpType.bypass,
                replica_groups=[list(range(N_CORES))],
                ins=[in_cc.opt()], outs=[out_cc.opt()])
            for j in range(6):
                nc.gpsimd.indirect_dma_start(
                    out=att_oth[:, j, :],
                    out_offset=None,
                    in_=out_cc[:],
                    in_offset=bass.IndirectOffsetOnAxis(
                        ap=meta_sb[:, j:j + 1], axis=0),
                )

            # ---- output projection (full rows) ----
            co_ranges = [(0, 512), (512, 768)]
            for tch in range(T // P):
                po = psc.tile([P, C], F32, tag="sc")
                for j in range(12):
                    src = att_all if j < 6 else att_oth
                    lhsT = src[:, j % 6, ts(tch, P)]
                    for (c0, c1) in co_ranges:
                        nc.tensor.matmul(po[:, c0:c1], lhsT,
                                         wpt[:, j, c0:c1],
                                         start=(j == 0), stop=False)
                for (c0, c1) in co_ranges:
                    nc.tensor.matmul(po[:, c0:c1], ones1[:],
                                     bp_sb[:, c0:c1], start=False, stop=True)
                ot = sbp.tile([P, C], F32, tag="out")
                nc.vector.tensor_copy(ot[:], po[:])
                nc.sync.dma_start(out_d[ts(tch, P), :], ot[:])

    nc.compile()
    return nc


_cached = {}


def get_nc(T=T_FULL):
    if T not in _cached:
        _cached[T] = build(T)
    return _cached[T]


def _make_in_maps(x, Wq, Wk, Wv, Wp, bp):
    scale = HS ** -0.5
    in_maps = []
    for c in range(N_CORES):
        b, s = c // 2, c % 2
        heads = list(range(6 * s, 6 * s + 6))
        xt = np.ascontiguousarray(np.asarray(x)[b].T).astype(bf16)
        def packw(W, sc=1.0):
            cols = []
            for rr in range(3):
                h0, h1 = heads[2 * rr], heads[2 * rr + 1]
                cols.append(np.concatenate([W[h0], W[h1]], axis=1))
            return (np.concatenate(cols, axis=1) * sc).astype(bf16)
        order = heads + [h for h in range(H) if h not in heads]
        wpt = np.concatenate(
            [np.asarray(Wp)[:, 64 * h:64 * h + 64].T for h in order],
            axis=0).astype(bf16)
        meta = ((c ^ 1) * 384 + 6 * np.arange(64, dtype=np.int32)[:, None]
                + np.arange(6, dtype=np.int32)[None, :]).astype(np.int32)
        in_maps.append({
            "xt": xt,
            "wq": packw(np.asarray(Wq), scale),
            "wk": packw(np.asarray(Wk)),
            "wv": packw(np.asarray(Wv)),
            "wpt": wpt,
            "bp": np.asarray(bp).reshape(1, C).astype(bf16),
            "meta": meta,
        })
    return in_maps


def kernel(x, Wq, Wk, Wv, Wp, bp):
    nc = get_nc(T_FULL)
    in_maps = _make_in_maps(x, Wq, Wk, Wv, Wp, bp)
    res = run_bass_kernel_spmd(nc, in_maps, list(range(N_CORES)))
    out = np.stack([res.results[2 * b]["out"] for b in range(B)])
    return out.astype(np.float32)


# revision 9
# speedup vs baseline: 1.1547x; 1.1547x over previous
"""Multi-head causal attention (B=4,T=2048,C=768,H=12,HS=64) on 8 trn2 cores.

Sharding: core c -> (batch b=c//2, head-half s=c%2, heads 6s..6s+5).
Each core computes QKV + attention for its 6 heads over the full T, then the
cores of a batch exchange attention outputs via a world AllGather and each
computes the full output projection for its batch.  Host keeps core 2b's
output for batch b.

All matmuls in bf16 (PSUM accumulates fp32).  Scores are computed
transposed ([tk, tq]) so softmax denominators come from a ones-column in V
and no PE transposes of the TxT attention matrix are needed.
"""
import sys

if "/opt/trn_rl_repo" not in sys.path:
    sys.path.insert(0, "/opt/trn_rl_repo")

import numpy as np
import ml_dtypes

import concourse.bass as bass
import concourse.mybir as mybir
import concourse.tile as tile
from concourse import bacc
from concourse.bass import ts, ds
from concourse.bass_utils import run_bass_kernel_spmd
from concourse.masks import make_identity

B, T_FULL, C, H, HS = 4, 2048, 768, 12, 64
N_CORES = 8
P = 128
NCH = C // P  # 6 c-chunks
dt = mybir.dt
bf16 = ml_dtypes.bfloat16
F32 = dt.float32
BF = dt.bfloat16


def build(T=T_FULL):
    nq = T // 512          # number of 512-wide q ranges
    nkt = T // P           # number of 128-wide k tiles
    nc = bacc.Bacc("TRN2", target_bir_lowering=False, debug=False,
                   num_devices=N_CORES)
    xt_d = nc.dram_tensor("xt", [C, T], BF, kind="ExternalInput").ap()
    wq_d = nc.dram_tensor("wq", [C, 384], BF, kind="ExternalInput").ap()
    wk_d = nc.dram_tensor("wk", [C, 384], BF, kind="ExternalInput").ap()
    wv_d = nc.dram_tensor("wv", [C, 384], BF, kind="ExternalInput").ap()
    wpt_d = nc.dram_tensor("wpt", [C, C], BF, kind="ExternalInput").ap()
    bp_d = nc.dram_tensor("bp", [1, C], BF, kind="ExternalInput").ap()
    meta_d = nc.dram_tensor("meta", [64, 6], dt.int32, kind="ExternalInput").ap()
    out_d = nc.dram_tensor("out", [T, C], F32, kind="ExternalOutput").ap()

    with tile.TileContext(nc) as tc:
        with tc.tile_pool(name="const", bufs=1) as const, \
             tc.tile_pool(name="big", bufs=1) as big, \
             tc.tile_pool(name="sbp", bufs=2) as sbp, \
             tc.tile_pool(name="ptp", bufs=3) as ptp, \
             tc.tile_pool(name="pp", bufs=2, space="PSUM") as pp, \
             tc.tile_pool(name="psc", bufs=2, space="PSUM") as psc, \
             tc.tile_pool(name="pso", bufs=2, space="PSUM") as pso, \
             tc.tile_pool(name="dram", bufs=1, space="DRAM") as dram:

            # ---- persistent loads ----
            xt = big.tile([P, NCH, T], BF, tag="xt")
            nc.sync.dma_start(xt[:], xt_d.rearrange("(n p) m -> p n m", p=P))
            wq = big.tile([P, NCH, 3, P], BF, tag="wq")
            wk = big.tile([P, NCH, 3, P], BF, tag="wk")
            wv = big.tile([P, NCH, 3, P], BF, tag="wv")
            for wt, wd in ((wq, wq_d), (wk, wk_d), (wv, wv_d)):
                nc.sync.dma_start(
                    wt[:], wd.rearrange("(n p) (r m) -> p n r m", p=P, m=P))
            wpt = big.tile([64, 12, C], BF, tag="wpt")
            nc.sync.dma_start(wpt[:], wpt_d.rearrange("(j p) m -> p j m", p=64))
            bp_sb = const.tile([1, C], BF, tag="bp")
            nc.sync.dma_start(bp_sb[:], bp_d[:])
            meta_sb = const.tile([64, 6], dt.int32, tag="meta")
            nc.sync.dma_start(meta_sb[:], meta_d[:])
            ones1 = const.tile([1, P], BF, tag="ones1")
            nc.gpsimd.memset(ones1[:], 1.0)
            ident = const.tile([P, P], BF, tag="ident")
            make_identity(nc, ident[:])

            att_all = big.tile([64, 6, T], BF, tag="att_all")
            att_oth = big.tile([64, 6, T], BF, tag="att_oth")

            EXP = mybir.ActivationFunctionType.Exp

            # ---- per head-pair: QKV + attention ----
            for r in range(3):
                qt = sbp.tile([P, T], BF, tag="qt")
                kt_t = sbp.tile([P, T], BF, tag="kt")
                vt_t = sbp.tile([P, T], BF, tag="vt")
                for wt, dst in ((wq, qt), (wk, kt_t), (wv, vt_t)):
                    for tr in range(nq):
                        ps = pp.tile([P, 512], F32, tag="qkv")
                        for ci in range(NCH):
                            nc.tensor.matmul(ps[:], wt[:, ci, r, :],
                                             xt[:, ci, ts(tr, 512)],
                                             start=(ci == 0), stop=(ci == NCH - 1))
                        nc.vector.tensor_copy(dst[:, ts(tr, 512)], ps[:])
                # V natural (+ ones col) per head
                vaug0 = sbp.tile([P, nkt, 128], BF, tag="vaug0")
                vaug1 = sbp.tile([P, nkt, 128], BF, tag="vaug1")
                nc.gpsimd.memset(vaug0[:, :, 64:128], 1.0)
                nc.gpsimd.memset(vaug1[:, :, 64:128], 1.0)
                for k in range(nkt):
                    pst = pp.tile([P, P], BF, tag="qkv")
                    nc.tensor.transpose(pst[:], vt_t[:, ts(k, P)], ident[:])
                    nc.vector.tensor_copy(vaug0[:, k, 0:64], pst[:, 0:64])
                    nc.vector.tensor_copy(vaug1[:, k, 0:64], pst[:, 64:128])
                vaug = (vaug0, vaug1)

                # attention per q-range, heads a=0,1 interleaved
                for qr in range(nq):
                    n_k = 4 * qr + 4       # causal: k tiles 0..4qr+3
                    oT0 = pso.tile([P, 512], F32, tag="oT")
                    oT1 = pso.tile([P, 512], F32, tag="oT")
                    oT = (oT0, oT1)
                    for k0 in range(0, n_k, 2):
                        scs, offs = [], []
                        for a in (0, 1):
                            sc = psc.tile([P, 2, 512], F32, tag="sc")
                            lo = 128 * max(0, k0 - 4 * qr)  # lowest col offset
                            for dk in (0, 1):
                                kti = k0 + dk
                                # cover from the pair's low offset so the
                                # whole exp'd region is written (extra cols
                                # are masked region, never read by PV)
                                nc.tensor.matmul(
                                    sc[:, dk, lo:512],
                                    kt_t[64 * a:64 * a + 64, ts(kti, P)],
                                    qt[64 * a:64 * a + 64,
                                       qr * 512 + lo:(qr + 1) * 512],
                                    start=True, stop=True)
                            scs.append(sc)
                            offs.append(lo)
                        pts = []
                        for a in (0, 1):
                            lo = offs[a]
                            pt = ptp.tile([P, 2, 512], BF, tag="pt")
                            nc.scalar.activation(pt[:, :, lo:512],
                                                 scs[a][:, :, lo:512], EXP)
                            for dk in (0, 1):
                                m = k0 + dk - 4 * qr
                                if 0 <= m < 4:
                                    blk = pt[:, dk, 128 * m:128 * (m + 1)]
                                    nc.gpsimd.affine_select(
                                        out=blk, in_=blk,
                                        compare_op=mybir.AluOpType.is_ge,
                                        fill=0.0, base=0,
                                        pattern=[[1, P]], channel_multiplier=-1)
                            pts.append(pt)
                        for a in (0, 1):
                            for dk in (0, 1):
                                kti = k0 + dk
                                off = 128 * max(0, kti - 4 * qr)
                                nc.tensor.matmul(
                                    oT[a][:, off:512],
                                    vaug[a][:, kti, :],
                                    pts[a][:, dk, off:512],
                                    start=(kti == 0), stop=(kti == n_k - 1))
                    for a in (0, 1):
                        rb = sbp.tile([64, 512], F32, tag="rb")
                        nc.vector.reciprocal(rb[:], oT[a][64:128, :])
                        nc.vector.tensor_mul(
                            att_all[:, 2 * r + a, ts(qr, 512)],
                            oT[a][0:64, :], rb[:])

            # ---- exchange attT halves via world AllGather ----
            # in_cc row index = p*6 + j so one indirect gather (64 rows of
            # 6*T) fetches the partner slab; indices are host-fed via meta.
            in_cc = dram.tile([384, T], BF, tag="in_cc")
            out_cc = dram.tile([N_CORES * 384, T], BF, tag="out_cc")
            nc.sync.dma_start(in_cc.rearrange("(p j) m -> p j m", j=6),
                              att_all[:])
            nc.gpsimd.collective_compute(
                "AllGather", mybir.AluOpType.bypass,
                replica_groups=[list(range(N_CORES))],
                ins=[in_cc.opt()], outs=[out_cc.opt()])
            for j in range(6):
                nc.gpsimd.indirect_dma_start(
                    out=att_oth[:, j, :],
                    out_offset=None,
                    in_=out_cc[:],
                    in_offset=bass.IndirectOffsetOnAxis(
                        ap=meta_sb[:, j:j + 1], axis=0),
                )

            # ---- output projection (full rows) ----
            co_ranges = [(0, 512), (512, 768)]
            for tch in range(T // P):
                po = psc.tile([P, C], F32, tag="sc")
                for j in range(12):
                    src = att_all if j < 6 else att_oth
                    lhsT = src[:, j % 6, ts(tch, P)]
                    for (c0, c1) in co_ranges:
                        nc.tensor.matmul(po[:, c0:c1], lhsT,
                                         wpt[:, j, c0:c1],
                                         start=(j == 0), stop=False)
                for (c0, c1) in co_ranges:
                    nc.tensor.matmul(po[:, c0:c1], ones1[:],
                                     bp_sb[:, c0:c1], start=False, stop=True)
                ot = sbp.tile([P, C], F32, tag="out")
                nc.vector.tensor_copy(ot[:], po[:])
                nc.sync.dma_start(out_d[ts(tch, P), :], ot[:])

    nc.compile()
    return nc


_cached = {}


def get_nc(T=T_FULL):
    if T not in _cached:
        _cached[T] = build(T)
    return _cached[T]


def _make_in_maps(x, Wq, Wk, Wv, Wp, bp):
    scale = HS ** -0.5
    in_maps = []
    for c in range(N_CORES):
        b, s = c // 2, c % 2
        heads = list(range(6 * s, 6 * s + 6))
        xt = np.ascontiguousarray(np.asarray(x)[b].T).astype(bf16)
        def packw(W, sc=1.0):
            cols = []
            for rr in range(3):
                h0, h1 = heads[2 * rr], heads[2 * rr + 1]
                cols.append(np.concatenate([W[h0], W[h1]], axis=1))
            return (np.concatenate(cols, axis=1) * sc).astype(bf16)
        order = heads + [h for h in range(H) if h not in heads]
        wpt = np.concatenate(
            [np.asarray(Wp)[:, 64 * h:64 * h + 64].T for h in order],
            axis=0).astype(bf16)
        meta = ((c ^ 1) * 384 + 6 * np.arange(64, dtype=np.int32)[:, None]
                + np.arange(6, dtype=np.int32)[None, :]).astype(np.int32)
        in_maps.append({
            "xt": xt,
            "wq": packw(np.asarray(Wq), scale),
            "wk": packw(np.asarray(Wk)),
            "wv": packw(np.asarray(Wv)),
            "wpt": wpt,
            "bp": np.asarray(bp).reshape(1, C).astype(bf16),
            "meta": meta,
        })
    return in_maps


def kernel(x, Wq, Wk, Wv, Wp, bp):
    nc = get_nc(T_FULL)
    in_maps = _make_in_maps(x, Wq, Wk, Wv, Wp, bp)
    res = run_bass_kernel_spmd(nc, in_maps, list(range(N_CORES)))
    out = np.stack([res.results[2 * b]["out"] for b in range(B)])
    return out.astype(np.float32)


# revision 12
# speedup vs baseline: 1.2010x; 1.0401x over previous
"""Multi-head causal attention (B=4,T=2048,C=768,H=12,HS=64) on 8 trn2 cores.

Sharding: core c -> (batch b=c//2, head-half s=c%2, heads 6s..6s+5).
Each core computes QKV + attention for its 6 heads over the full T, then the
cores of a batch exchange attention outputs via a world AllGather and each
computes the full output projection for its batch.  Host keeps core 2b's
output for batch b.

All matmuls in bf16 (PSUM accumulates fp32).  Scores are computed
transposed ([tk, tq]) so softmax denominators come from a ones-column in V
and no PE transposes of the TxT attention matrix are needed.
"""
import sys

if "/opt/trn_rl_repo" not in sys.path:
    sys.path.insert(0, "/opt/trn_rl_repo")

import numpy as np
import ml_dtypes

import concourse.bass as bass
import concourse.mybir as mybir
import concourse.tile as tile
from concourse import bacc
from concourse.bass import ts, ds
from concourse.bass_utils import run_bass_kernel_spmd
from concourse.masks import make_identity

B, T_FULL, C, H, HS = 4, 2048, 768, 12, 64
N_CORES = 8
P = 128
NCH = C // P  # 6 c-chunks
dt = mybir.dt
bf16 = ml_dtypes.bfloat16
F32 = dt.float32
BF = dt.bfloat16


def build(T=T_FULL):
    nq = T // 512          # number of 512-wide q ranges
    nkt = T // P           # number of 128-wide k tiles
    nc = bacc.Bacc("TRN2", target_bir_lowering=False, debug=False,
                   num_devices=N_CORES)
    xt_d = nc.dram_tensor("xt", [C, T], BF, kind="ExternalInput").ap()
    wq_d = nc.dram_tensor("wq", [C, 384], BF, kind="ExternalInput").ap()
    wk_d = nc.dram_tensor("wk", [C, 384], BF, kind="ExternalInput").ap()
    wv_d = nc.dram_tensor("wv", [C, 384], BF, kind="ExternalInput").ap()
    wpt_d = nc.dram_tensor("wpt", [C, C], BF, kind="ExternalInput").ap()
    bp_d = nc.dram_tensor("bp", [1, C], BF, kind="ExternalInput").ap()
    meta_d = nc.dram_tensor("meta", [64, 2], dt.int32, kind="ExternalInput").ap()
    out_d = nc.dram_tensor("out", [T, C], F32, kind="ExternalOutput").ap()

    with tile.TileContext(nc) as tc:
        with tc.tile_pool(name="const", bufs=1) as const, \
             tc.tile_pool(name="big", bufs=1) as big, \
             tc.tile_pool(name="sbp", bufs=2) as sbp, \
             tc.tile_pool(name="ptp", bufs=3) as ptp, \
             tc.tile_pool(name="pp", bufs=2, space="PSUM") as pp, \
             tc.tile_pool(name="psc", bufs=2, space="PSUM") as psc, \
             tc.tile_pool(name="pso", bufs=2, space="PSUM") as pso, \
             tc.tile_pool(name="dram", bufs=1, space="DRAM") as dram:

            # ---- persistent loads ----
            xt = big.tile([P, NCH, T], BF, tag="xt")
            nc.sync.dma_start(xt[:], xt_d.rearrange("(n p) m -> p n m", p=P))
            wq = big.tile([P, NCH, 3, P], BF, tag="wq")
            wk = big.tile([P, NCH, 3, P], BF, tag="wk")
            wv = big.tile([P, NCH, 3, P], BF, tag="wv")
            for wt, wd in ((wq, wq_d), (wk, wk_d), (wv, wv_d)):
                nc.sync.dma_start(
                    wt[:], wd.rearrange("(n p) (r m) -> p n r m", p=P, m=P))
            wpt = big.tile([64, 12, C], BF, tag="wpt")
            nc.sync.dma_start(wpt[:], wpt_d.rearrange("(j p) m -> p j m", p=64))
            bp_sb = const.tile([1, C], BF, tag="bp")
            nc.sync.dma_start(bp_sb[:], bp_d[:])
            meta_sb = const.tile([64, 2], dt.int32, tag="meta")
            nc.sync.dma_start(meta_sb[:], meta_d[:])
            ones1 = const.tile([1, P], BF, tag="ones1")
            nc.gpsimd.memset(ones1[:], 1.0)
            ident = const.tile([P, P], BF, tag="ident")
            make_identity(nc, ident[:])

            att_all = big.tile([64, 6, T], BF, tag="att_all")
            att_oth = big.tile([64, 6, T], BF, tag="att_oth")

            EXP = mybir.ActivationFunctionType.Exp

            # ---- per head-pair: QKV + attention ----
            for r in range(3):
                qt = sbp.tile([P, T], BF, tag="qt")
                kt_t = sbp.tile([P, T], BF, tag="kt")
                vt_t = sbp.tile([P, T], BF, tag="vt")
                for wt, dst in ((wq, qt), (wk, kt_t), (wv, vt_t)):
                    for tr in range(nq):
                        ps = pp.tile([P, 512], F32, tag="qkv")
                        for ci in range(NCH):
                            nc.tensor.matmul(ps[:], wt[:, ci, r, :],
                                             xt[:, ci, ts(tr, 512)],
                                             start=(ci == 0), stop=(ci == NCH - 1))
                        nc.vector.tensor_copy(dst[:, ts(tr, 512)], ps[:])
                # V natural (+ ones col) per head
                vaug0 = sbp.tile([P, nkt, 128], BF, tag="vaug0")
                vaug1 = sbp.tile([P, nkt, 128], BF, tag="vaug1")
                nc.gpsimd.memset(vaug0[:, :, 64:128], 1.0)
                nc.gpsimd.memset(vaug1[:, :, 64:128], 1.0)
                for k in range(nkt):
                    pst = pp.tile([P, P], BF, tag="qkv")
                    nc.tensor.transpose(pst[:], vt_t[:, ts(k, P)], ident[:])
                    nc.vector.tensor_copy(vaug0[:, k, 0:64], pst[:, 0:64])
                    nc.vector.tensor_copy(vaug1[:, k, 0:64], pst[:, 64:128])
                vaug = (vaug0, vaug1)

                # attention per q-range, heads a=0,1 interleaved
                for qr in range(nq):
                    n_k = 4 * qr + 4       # causal: k tiles 0..4qr+3
                    oT0 = pso.tile([P, 512], F32, tag="oT")
                    oT1 = pso.tile([P, 512], F32, tag="oT")
                    oT = (oT0, oT1)
                    for k0 in range(0, n_k, 2):
                        scs, offs = [], []
                        for a in (0, 1):
                            sc = psc.tile([P, 2, 512], F32, tag="sc")
                            lo = 128 * max(0, k0 - 4 * qr)  # lowest col offset
                            for dk in (0, 1):
                                kti = k0 + dk
                                # cover from the pair's low offset so the
                                # whole exp'd region is written (extra cols
                                # are masked region, never read by PV)
                                nc.tensor.matmul(
                                    sc[:, dk, lo:512],
                                    kt_t[64 * a:64 * a + 64, ts(kti, P)],
                                    qt[64 * a:64 * a + 64,
                                       qr * 512 + lo:(qr + 1) * 512],
                                    start=True, stop=True)
                            scs.append(sc)
                            offs.append(lo)
                        pts = []
                        for a in (0, 1):
                            lo = offs[a]
                            pt = ptp.tile([P, 2, 512], BF, tag="pt")
                            nc.scalar.activation(pt[:, :, lo:512],
                                                 scs[a][:, :, lo:512], EXP)
                            for dk in (0, 1):
                                m = k0 + dk - 4 * qr
                                if 0 <= m < 4:
                                    blk = pt[:, dk, 128 * m:128 * (m + 1)]
                                    nc.gpsimd.affine_select(
                                        out=blk, in_=blk,
                                        compare_op=mybir.AluOpType.is_ge,
                                        fill=0.0, base=0,
                                        pattern=[[1, P]], channel_multiplier=-1)
                            pts.append(pt)
                        for a in (0, 1):
                            for dk in (0, 1):
                                kti = k0 + dk
                                off = 128 * max(0, kti - 4 * qr)
                                nc.tensor.matmul(
                                    oT[a][:, off:512],
                                    vaug[a][:, kti, :],
                                    pts[a][:, dk, off:512],
                                    start=(kti == 0), stop=(kti == n_k - 1))
                    for a in (0, 1):
                        rb = sbp.tile([64, 512], F32, tag="rb")
                        nc.vector.reciprocal(rb[:], oT[a][64:128, :])
                        nc.vector.tensor_mul(
                            att_all[:, 2 * r + a, ts(qr, 512)],
                            oT[a][0:64, :], rb[:])

                # pipelined exchange: AllGather this pair's slab while later
                # pairs still compute.  in_cc_r row index = p*2 + a.
                in_cc_r = dram.tile([P, T], BF, tag=f"in_cc{r}")
                out_cc_r = dram.tile([N_CORES * P, T], BF, tag=f"out_cc{r}")
                nc.sync.dma_start(in_cc_r.rearrange("(p j) m -> p j m", j=2),
                                  att_all[:, 2 * r:2 * r + 2, :])
                nc.gpsimd.collective_compute(
                    "AllGather", mybir.AluOpType.bypass,
                    replica_groups=[list(range(N_CORES))],
                    ins=[in_cc_r.opt()], outs=[out_cc_r.opt()])
                for a in (0, 1):
                    nc.gpsimd.indirect_dma_start(
                        out=att_oth[:, 2 * r + a, :],
                        out_offset=None,
                        in_=out_cc_r[:],
                        in_offset=bass.IndirectOffsetOnAxis(
                            ap=meta_sb[:, a:a + 1], axis=0),
                    )

            # ---- output projection (full rows) ----
            co_ranges = [(0, 512), (512, 768)]
            for tch in range(T // P):
                po = psc.tile([P, C], F32, tag="sc")
                for j in range(12):
                    src = att_all if j < 6 else att_oth
                    lhsT = src[:, j % 6, ts(tch, P)]
                    for (c0, c1) in co_ranges:
                        nc.tensor.matmul(po[:, c0:c1], lhsT,
                                         wpt[:, j, c0:c1],
                                         start=(j == 0), stop=False)
                for (c0, c1) in co_ranges:
                    nc.tensor.matmul(po[:, c0:c1], ones1[:],
                                     bp_sb[:, c0:c1], start=False, stop=True)
                ot = sbp.tile([P, C], F32, tag="out")
                nc.vector.tensor_copy(ot[:], po[:])
                nc.sync.dma_start(out_d[ts(tch, P), :], ot[:])

    nc.compile()
    return nc


_cached = {}


def get_nc(T=T_FULL):
    if T not in _cached:
        _cached[T] = build(T)
    return _cached[T]


def _make_in_maps(x, Wq, Wk, Wv, Wp, bp):
    scale = HS ** -0.5
    in_maps = []
    for c in range(N_CORES):
        b, s = c // 2, c % 2
        heads = list(range(6 * s, 6 * s + 6))
        xt = np.ascontiguousarray(np.asarray(x)[b].T).astype(bf16)
        def packw(W, sc=1.0):
            cols = []
            for rr in range(3):
                h0, h1 = heads[2 * rr], heads[2 * rr + 1]
                cols.append(np.concatenate([W[h0], W[h1]], axis=1))
            return (np.concatenate(cols, axis=1) * sc).astype(bf16)
        order = heads + [h for h in range(H) if h not in heads]
        wpt = np.concatenate(
            [np.asarray(Wp)[:, 64 * h:64 * h + 64].T for h in order],
            axis=0).astype(bf16)
        meta = ((c ^ 1) * 128 + 2 * np.arange(64, dtype=np.int32)[:, None]
                + np.arange(2, dtype=np.int32)[None, :]).astype(np.int32)
        in_maps.append({
            "xt": xt,
            "wq": packw(np.asarray(Wq), scale),
            "wk": packw(np.asarray(Wk)),
            "wv": packw(np.asarray(Wv)),
            "wpt": wpt,
            "bp": np.asarray(bp).reshape(1, C).astype(bf16),
            "meta": meta,
        })
    return in_maps


def kernel(x, Wq, Wk, Wv, Wp, bp):
    nc = get_nc(T_FULL)
    in_maps = _make_in_maps(x, Wq, Wk, Wv, Wp, bp)
    res = run_bass_kernel_spmd(nc, in_maps, list(range(N_CORES)))
    out = np.stack([res.results[2 * b]["out"] for b in range(B)])
    return out.astype(np.float32)
